# revision 1
# baseline (speedup 1.0000x reference)
"""Trainium2 Bass kernel for nn_BinarySquareClassifier (3-layer LIF SNN).

Strategy (pure data parallel over batch, 8 cores, B=2048 -> 256/core):
- One stacked f32 matmul per 8-step time-chunk computes h1/h2/h3 for all
  three layers at once: lhsT [128, 98] holds W2.T/W3.T against the spike
  rows (rhs rows 0:96) and W1.T against the x rows (rhs rows 98:128);
  layer l's inputs come one chunk later than layer l-1's outputs (pipeline
  skew), so the serial LIF scans of the three layers run on time-shifted
  frames and stack into one [98, 256] membrane state M.
- Biases are folded away with the shift m^ = m - b/(1-beta): per-partition
  spike thresholds th = 1 - b/(1-beta), zero-input warmup freezing via +BIG
  thresholds and pre-decayed initial states.
- Per scan step, software-pipelined over two batch halves on DVE
  (u = beta*M + H then M = u - r, the halves' ops interleaved to cover
  semaphore latency); the spike op r = (M > th) runs on the Pool engine
  and doubles as the spike row written into the next chunk's matmul rhs.
  The ACT engine copies each PSUM matmul block to SBUF for cheaper reads.
- Layer-3 spike rows (rhs rows 96:98) are stashed to a [128, 4096] SBUF
  tile via SBUF->SBUF DMA (partition = time) and reduced at the end with
  tensor adds + a ones-vector matmul over partitions.
"""

import numpy as np
from contextlib import ExitStack

B_FULL, C_IN, T_FULL = 2048, 30, 1024
N_CORES = 8
B = B_FULL // N_CORES           # 256 batch per core
TC = 8                          # timesteps per chunk
N_CHUNKS = T_FULL // TC         # 128
BETA = 0.9
BIG = 3.0e38

_cache = {}


def _split_multi_waits(nc):
    """This container's walrus accepts only ONE sync-wait per instruction;
    hoist extra waits onto same-engine NoOps inserted just before."""
    import concourse.mybir as mybir
    counter = 0
    for f in nc.m.functions:
        for blk in f.blocks:
            out = []
            changed = False
            for inst in blk.instructions:
                si = inst.sync_info
                if si is not None and si.on_wait is not None and len(si.on_wait) > 1:
                    waits = list(si.on_wait)
                    for w in waits[:-1]:
                        counter += 1
                        nop = mybir.InstNoOp(
                            name=f"waitsplit-{counter}", ins=[], outs=[])
                        nop.engine = inst.engine
                        nop.sync_info = mybir.SyncInfo(on_wait=[w], on_update=[])
                        out.append(nop)
                    inst.sync_info = mybir.SyncInfo(
                        on_wait=[waits[-1]], on_update=list(si.on_update or []))
                    changed = True
                out.append(inst)
            if changed:
                try:
                    blk.instructions[:] = out
                except TypeError:
                    blk.instructions = out


def _build_program(SP=B):
    import concourse.bass as bass
    import concourse.mybir as mybir
    import concourse.tile as tile

    nc = bass.Bass("TRN2", target_bir_lowering=False, debug=False,
                   num_devices=N_CORES)
    dt = mybir.dt.float32
    AOT = mybir.AluOpType

    xt_in = nc.dram_tensor("xt", [C_IN, T_FULL, B], dt, kind="ExternalInput").ap()
    lhsT_in = nc.dram_tensor("lhsT", [128, 98], dt, kind="ExternalInput").ap()
    th_in = nc.dram_tensor("th", [98, 3], dt, kind="ExternalInput").ap()
    m0_in = nc.dram_tensor("m0", [98, B], dt, kind="ExternalInput").ap()
    acc_out = nc.dram_tensor("acc", [1, 512], dt, kind="ExternalOutput").ap()

    with ExitStack() as ctx:
        tc = ctx.enter_context(tile.TileContext(nc))
        pool = ctx.enter_context(tc.tile_pool(name="sb", bufs=1))
        psum_pool = ctx.enter_context(tc.tile_pool(name="ps", bufs=1, space="PSUM"))

        t_lhsT = pool.tile([128, 98], dt, tag="lhsT", name="lhsT")
        t_th = pool.tile([98, 3], dt, tag="th", name="th")
        t_M = pool.tile([98, B], dt, tag="M", name="M")
        t_u = pool.tile([98, B], dt, tag="u", name="u")
        t_rhs = [pool.tile([128, TC * B], dt, tag=f"rhs{i}", name=f"rhs{i}") for i in range(2)]
        t_stash = pool.tile([128, 4096], dt, tag="stash", name="stash")
        t_ones = pool.tile([128, 1], dt, tag="ones", name="ones")
        t_part = pool.tile([128, 512], dt, tag="part", name="part")
        t_accf = pool.tile([1, 512], dt, tag="accf", name="accf")
        t_ps = [psum_pool.tile([98, TC * B], dt, tag=f"H{i}", name=f"H{i}") for i in range(2)]
        t_hs = [pool.tile([98, TC * B], dt, tag=f"Hs{i}", name=f"Hs{i}") for i in range(2)]
        t_psr = t_ps[1][0:1, 0:512]  # reuse a PSUM bank for the final reduce

        nc.sync.dma_start(out=t_lhsT[:], in_=lhsT_in[:])
        nc.sync.dma_start(out=t_th[:], in_=th_in[:])
        nc.sync.dma_start(out=t_M[:], in_=m0_in[:])
        nc.gpsimd.memset(t_ones[:], 1.0)
        # Only rhs0's spike rows need zeroing (rhs1's are fully written by
        # chunk 0's scan before anything reads them; rows 0:98 also cover
        # the tau=0 reset read of the last column). Split per matmul block
        # so the first sub-matmul isn't gated on the whole memset.
        for blk in range(4):
            nc.gpsimd.memset(t_rhs[0][0:98, blk * 512:(blk + 1) * 512], 0.0)

        # x prefetch for chunk 0, split per matmul block so the first
        # sub-matmul starts as soon as its two t-columns have landed
        for blk in range(4):
            nc.sync.dma_start(
                out=t_rhs[0][98:128, blk * 512:(blk + 1) * 512],
                in_=xt_in[:, blk * 2:(blk + 1) * 2, :].rearrange(
                    "c t b -> c (t b)"),
            )

        def th_col(c):
            return 0 if c == 0 else (1 if c == 1 else 2)

        for c in range(N_CHUNKS + 2):
            cur = t_rhs[c % 2]
            nxt = t_rhs[(c + 1) % 2]
            ps = t_ps[c % 2]
            th = t_th[:, th_col(c):th_col(c) + 1]

            # prefetch x for chunk c+1 (overlaps this chunk's scan)
            if c + 1 < N_CHUNKS:
                nc.sync.dma_start(
                    out=nxt[98:128, :],
                    in_=xt_in[:, (c + 1) * TC:(c + 2) * TC, :].rearrange(
                        "c t b -> c (t b)"),
                )

            # stacked matmul for this chunk, 4 sub-matmuls of 512 columns,
            # each copied PSUM->SBUF by the (otherwise idle) ACT engine
            hs = t_hs[c % 2]
            for blk in range(4):
                sl = slice(blk * 512, (blk + 1) * 512)
                nc.tensor.matmul(ps[:, sl], t_lhsT[:], cur[0:128, sl])
                nc.scalar.copy(hs[:, sl], ps[:, sl])

            # serial LIF scan. Columns [0:SP) run on DVE, columns [SP:B) on
            # the Pool engine -- each engine software-pipelines two column
            # sub-slices so every same-engine sem hop is covered by the other
            # slice's op. Spike ops all run on Pool.
            for tau in range(TC):
                if tau == 0:
                    r_ap = cur[0:98, (TC - 1) * B:TC * B]
                else:
                    r_ap = nxt[0:98, (tau - 1) * B:tau * B]
                h0 = tau * B

                def stt(eng, lo, hi):
                    eng.scalar_tensor_tensor(
                        t_u[:, lo:hi], t_M[:, lo:hi], BETA,
                        hs[:, h0 + lo:h0 + hi], AOT.mult, AOT.add)

                def tt(eng, lo, hi):
                    eng.tensor_tensor(
                        t_M[:, lo:hi], t_u[:, lo:hi], r_ap[:, lo:hi],
                        AOT.subtract)

                def spike(eng, lo, hi):
                    eng.tensor_scalar(
                        nxt[0:98, tau * B + lo:tau * B + hi],
                        t_M[:, lo:hi], th, None, AOT.is_gt)

                HA = SP // 2
                stt(nc.vector, 0, HA)
                stt(nc.vector, HA, SP)
                if SP < B:
                    PH = (B - SP) // 2
                    stt(nc.gpsimd, SP, SP + PH)
                    stt(nc.gpsimd, SP + PH, B)
                tt(nc.vector, 0, HA)
                tt(nc.vector, HA, SP)
                if SP < B:
                    tt(nc.gpsimd, SP, SP + PH)
                    tt(nc.gpsimd, SP + PH, B)
                # spike s(tau) = (M > th) -> nxt col tau (doubles as the
                # boundary column when tau == TC-1)
                spike(nc.gpsimd, 0, HA)
                spike(nc.gpsimd, HA, SP)
                if SP < B:
                    spike(nc.gpsimd, SP, B)

            # stash layer-3 spike rows for frame f = c-2
            f = c - 2
            if 0 <= f < N_CHUNKS:
                p0 = (f % 16) * TC
                cb = f // 16
                for j in range(2):
                    nc.sync.dma_start(
                        out=t_stash[p0:p0 + TC,
                                    cb * 512 + j * B:cb * 512 + (j + 1) * B],
                        in_=nxt[96 + j:97 + j, :],
                    )

        # reduce stash: sum the 8 column blocks, then sum over partitions
        nc.vector.tensor_tensor(
            t_part[:], t_stash[:, 0:512], t_stash[:, 512:1024], AOT.add)
        for cb in range(2, 8):
            nc.vector.tensor_tensor(
                t_part[:], t_part[:], t_stash[:, cb * 512:(cb + 1) * 512], AOT.add)
        nc.tensor.matmul(t_psr[:], t_ones[:], t_part[:])
        nc.scalar.copy(t_accf[:], t_psr[:])
        nc.gpsimd.dma_start(out=acc_out[:], in_=t_accf[:])

    _split_multi_waits(nc)
    return nc


def _host_consts(W1, b1, W2, b2, W3, b3):
    lhsT = np.zeros((128, 98), np.float32)
    lhsT[98:128, 0:64] = W1.T
    lhsT[0:64, 64:96] = W2.T
    lhsT[64:96, 96:98] = W3.T
    c1 = (b1.astype(np.float64) / (1.0 - BETA)).astype(np.float32)
    c2 = (b2.astype(np.float64) / (1.0 - BETA)).astype(np.float32)
    c3 = (b3.astype(np.float64) / (1.0 - BETA)).astype(np.float32)
    th_main = np.concatenate([
        (1.0 - c1.astype(np.float64)).astype(np.float32),
        (1.0 - c2.astype(np.float64)).astype(np.float32),
        (1.0 - c3.astype(np.float64)).astype(np.float32),
    ]).astype(np.float32)
    th_w0 = th_main.copy()
    th_w0[64:98] = BIG
    th_w1 = th_main.copy()
    th_w1[96:98] = BIG
    th = np.stack([th_w0, th_w1, th_main], axis=1)  # [98, 3]
    beta64 = np.float64(np.float32(BETA))
    m0 = np.zeros(98, np.float32)
    m0[0:64] = -c1
    m0[64:96] = (-c2.astype(np.float64) / beta64 ** TC).astype(np.float32)
    m0[96:98] = (-c3.astype(np.float64) / beta64 ** (2 * TC)).astype(np.float32)
    m0b = np.ascontiguousarray(
        np.broadcast_to(m0[:, None], (98, B))).astype(np.float32)
    return lhsT, th, m0b


def kernel(x, W1, b1, W2, b2, W3, b3):
    from concourse.bass_utils import run_bass_kernel_spmd

    x = np.asarray(x, np.float32)
    W1 = np.asarray(W1, np.float32); b1 = np.asarray(b1, np.float32)
    W2 = np.asarray(W2, np.float32); b2 = np.asarray(b2, np.float32)
    W3 = np.asarray(W3, np.float32); b3 = np.asarray(b3, np.float32)

    if "nc" not in _cache:
        _cache["nc"] = _build_program()
    nc = _cache["nc"]

    lhsT, th, m0b = _host_consts(W1, b1, W2, b2, W3, b3)
    in_maps = []
    for core in range(N_CORES):
        xs = x[core * B:(core + 1) * B]                # [256, 30, 1024]
        xt = np.ascontiguousarray(np.transpose(xs, (1, 2, 0)))  # [30, 1024, 256]
        in_maps.append({"xt": xt, "lhsT": lhsT, "th": th, "m0": m0b})

    res = run_bass_kernel_spmd(nc, in_maps, list(range(N_CORES)))
    out = np.empty((B_FULL, 2), np.float32)
    for core in range(N_CORES):
        a = res.results[core]["acc"].reshape(2, 256)   # [j, b]
        out[core * B:(core + 1) * B] = a.T
    return out



# revision 2
# speedup vs baseline: 1.0186x; 1.0186x over previous
"""Trainium2 Bass kernel for nn_BinarySquareClassifier (3-layer LIF SNN).

Baseline skeleton (stacked 3-layer matmul, chunk-skew pipeline, serial
in-chunk scan) with a window-inflated fp16 scan:

- Within each 8-step chunk the membrane is kept inflated: Mt = beta^(-j)*M
  at in-chunk phase j. The decay then disappears from the recurrence:
      u(j)  = Mt(j-1) + Ht(j)          (tensor_tensor add, fp16, 2x)
      Mt(j) = u(j) - st(j-1)           (tensor_tensor sub, fp16, 2x)
      st(j) = (Mt(j) > thr_j) * imm_j  (tensor_scalar 2-op, fp16, 4x/Pool)
  where Ht = beta^(-j)*H comes out of the matmul directly (x staged with
  beta^(-tau) prescale; spike rows carry their own scale; lhsT absorbs
  the rest), thr_j = beta^(-j)*thr, and imm_j = beta^(-(j+1)) scales the
  written spike so the NEXT step's plain subtract applies the correctly
  inflated reset (j=7 writes scale 1.0 for the phase-0 consumer).
- At each chunk boundary the roll Mt *= beta^8 is folded into the tau=0
  update as a scalar_tensor_tensor (u = beta^8*Mt + Ht).
- The spike value also feeds the next chunk's matmul: W2/W3 lhsT blocks
  absorb the written scale (x beta/TH for tau<=6 consumers, x beta^-7/TH
  for the tau=7 column), so two lhsT variants cover a chunk.
- acc: layer-3 spike rows are stashed per chunk and reduced with a
  weighted-ones matmul that divides out the per-phase spike scales.
"""

import numpy as np
from contextlib import ExitStack

B_FULL, C_IN, T_FULL = 2048, 30, 1024
N_CORES = 8
B = B_FULL // N_CORES           # 256 batch per core
TC = 8                          # timesteps per chunk = inflation window
N_CHUNKS = T_FULL // TC         # 128
BETA = 0.9
BIG = 3.0e4                     # fp16-safe "never fires" threshold

_cache = {}


def _split_multi_waits(nc):
    """This container's walrus accepts only ONE sync-wait per instruction;
    hoist extra waits onto same-engine NoOps inserted just before."""
    import concourse.mybir as mybir
    counter = 0
    for f in nc.m.functions:
        for blk in f.blocks:
            out = []
            changed = False
            for inst in blk.instructions:
                si = inst.sync_info
                if si is not None and si.on_wait is not None and len(si.on_wait) > 1:
                    waits = list(si.on_wait)
                    for w in waits[:-1]:
                        counter += 1
                        nop = mybir.InstNoOp(
                            name=f"waitsplit-{counter}", ins=[], outs=[])
                        nop.engine = inst.engine
                        nop.sync_info = mybir.SyncInfo(on_wait=[w], on_update=[])
                        out.append(nop)
                    inst.sync_info = mybir.SyncInfo(
                        on_wait=[waits[-1]], on_update=list(si.on_update or []))
                    changed = True
                out.append(inst)
            if changed:
                try:
                    blk.instructions[:] = out
                except TypeError:
                    blk.instructions = out


def _build_program():
    import concourse.bass as bass
    import concourse.mybir as mybir
    import concourse.tile as tile

    nc = bass.Bass("TRN2", target_bir_lowering=False, debug=False,
                   num_devices=N_CORES)
    f32 = mybir.dt.float32
    f16 = mybir.dt.float16
    AOT = mybir.AluOpType
    HA = B // 2

    binv = 1.0 / np.float64(np.float32(BETA))
    # spike write scale: written at phase j, consumed (as reset) at phase
    # j+1 (j<7) or phase 0 of the next chunk (j=7)
    imm = [float(np.float32(binv ** (j + 1))) if j < 7 else 1.0
           for j in range(TC)]

    xt_in = nc.dram_tensor("xt", [C_IN, T_FULL, B], f16, kind="ExternalInput").ap()
    lhsT_in = nc.dram_tensor("lhsT", [128, 2 * 98], f16, kind="ExternalInput").ap()
    th_in = nc.dram_tensor("th", [98, 3 * TC], f32, kind="ExternalInput").ap()
    m0_in = nc.dram_tensor("m0", [98, B], f16, kind="ExternalInput").ap()
    wred_in = nc.dram_tensor("wred", [128, 1], f16, kind="ExternalInput").ap()
    acc_out = nc.dram_tensor("acc", [1, 512], f32, kind="ExternalOutput").ap()

    with ExitStack() as ctx:
        tc = ctx.enter_context(tile.TileContext(nc))
        pool = ctx.enter_context(tc.tile_pool(name="sb", bufs=1))
        psum_pool = ctx.enter_context(tc.tile_pool(name="ps", bufs=1, space="PSUM"))

        t_lhsT = pool.tile([128, 2 * 98], f16, tag="lhsT", name="lhsT")
        t_th = pool.tile([98, 3 * TC], f32, tag="th", name="th")
        t_M = pool.tile([98, B], f16, tag="M", name="M")
        t_u = pool.tile([98, B], f16, tag="u", name="u")
        t_rhs = [pool.tile([128, TC * B], f16, tag=f"rhs{i}", name=f"rhs{i}")
                 for i in range(2)]
        t_stash = pool.tile([128, 4096], f16, tag="stash", name="stash")
        t_wred = pool.tile([128, 1], f16, tag="wred", name="wred")
        t_part = pool.tile([128, 512], f16, tag="part", name="part")
        t_accf = pool.tile([1, 512], f32, tag="accf", name="accf")
        t_ps = [psum_pool.tile([98, TC * B], f32, tag=f"H{i}", name=f"H{i}")
                for i in range(2)]
        t_hs = [pool.tile([98, TC * B], f16, tag=f"Hs{i}", name=f"Hs{i}")
                for i in range(2)]
        t_psr = t_ps[1][0:1, 0:512]  # reuse a PSUM bank for the final reduce

        nc.sync.dma_start(out=t_lhsT[:], in_=lhsT_in[:])
        nc.sync.dma_start(out=t_th[:], in_=th_in[:])
        nc.sync.dma_start(out=t_M[:], in_=m0_in[:])
        nc.sync.dma_start(out=t_wred[:], in_=wred_in[:])
        # Only rhs0's spike rows need zeroing (rhs1's are fully written by
        # chunk 0's scan before anything reads them; rows 0:98 also cover
        # the tau=0 reset read of the last column). Split per matmul block
        # so the first sub-matmul isn't gated on the whole memset.
        for blk in range(4):
            nc.gpsimd.memset(t_rhs[0][0:98, blk * 512:(blk + 1) * 512], 0.0)

        # x prefetch for chunk 0, split per matmul block
        for blk in range(4):
            nc.sync.dma_start(
                out=t_rhs[0][98:128, blk * 512:(blk + 1) * 512],
                in_=xt_in[:, blk * 2:(blk + 1) * 2, :].rearrange(
                    "c t b -> c (t b)"),
            )

        def th_group(c):
            return 0 if c == 0 else (1 if c == 1 else 2)

        for c in range(N_CHUNKS + 2):
            cur = t_rhs[c % 2]
            nxt = t_rhs[(c + 1) % 2]
            ps = t_ps[c % 2]
            hs = t_hs[c % 2]
            tg = th_group(c)

            # prefetch x for chunk c+1 (overlaps this chunk's scan)
            if c + 1 < N_CHUNKS:
                nc.sync.dma_start(
                    out=nxt[98:128, :],
                    in_=xt_in[:, (c + 1) * TC:(c + 2) * TC, :].rearrange(
                        "c t b -> c (t b)"),
                )

            # stacked matmul: taus 0..6 with lhsT variant A, tau 7 with
            # variant B (its spike rows carry the boundary scale), each
            # block copied PSUM->SBUF(fp16) by the ACT engine
            mm_blocks = [(0, 512, 0), (512, 1024, 0), (1024, 1536, 0),
                         (1536, 1792, 0), (1792, 2048, 98)]
            for lo, hi, lv in mm_blocks:
                nc.tensor.matmul(ps[:, lo:hi], t_lhsT[:, lv:lv + 98],
                                 cur[0:128, lo:hi])
                nc.scalar.copy(hs[:, lo:hi], ps[:, lo:hi])

            # serial inflated scan
            for tau in range(TC):
                if tau == 0:
                    r_ap = cur[0:98, (TC - 1) * B:TC * B]
                else:
                    r_ap = nxt[0:98, (tau - 1) * B:tau * B]
                h0 = tau * B
                th = t_th[:, tg * TC + tau:tg * TC + tau + 1]

                if tau == 0:
                    # chunk roll folded in: u = roll*Mt + Ht
                    roll = float(np.float32(BETA)) if c == 0 else \
                        float(np.float32(np.float64(np.float32(BETA)) ** TC))
                    def upd(lo, hi):
                        nc.vector.scalar_tensor_tensor(
                            t_u[:, lo:hi], t_M[:, lo:hi], roll,
                            hs[:, h0 + lo:h0 + hi], AOT.mult, AOT.add)
                else:
                    def upd(lo, hi):
                        nc.vector.tensor_tensor(
                            t_u[:, lo:hi], t_M[:, lo:hi],
                            hs[:, h0 + lo:h0 + hi], AOT.add)

                def tt(lo, hi):
                    nc.vector.tensor_tensor(
                        t_M[:, lo:hi], t_u[:, lo:hi], r_ap[:, lo:hi],
                        AOT.subtract)

                def spike(eng, lo, hi):
                    eng.tensor_scalar(
                        nxt[0:98, tau * B + lo:tau * B + hi],
                        t_M[:, lo:hi], th, imm[tau], AOT.is_gt, AOT.mult)

                upd(0, HA)
                upd(HA, B)
                tt(0, HA)
                tt(HA, B)
                spike(nc.vector, 0, B)

            # stash layer-3 spike rows for frame f = c-2
            f = c - 2
            if 0 <= f < N_CHUNKS:
                p0 = (f % 16) * TC
                cb = f // 16
                for jr in range(2):
                    nc.sync.dma_start(
                        out=t_stash[p0:p0 + TC,
                                    cb * 512 + jr * B:cb * 512 + (jr + 1) * B],
                        in_=nxt[96 + jr:97 + jr, :],
                    )

        # reduce stash: sum the 8 column blocks (fp32 accumulate), then a
        # weighted-ones matmul divides out the per-phase spike scales
        nc.vector.tensor_tensor(
            t_part[:], t_stash[:, 0:512], t_stash[:, 512:1024], AOT.add)
        for cb in range(2, 8):
            nc.vector.tensor_tensor(
                t_part[:], t_part[:], t_stash[:, cb * 512:(cb + 1) * 512],
                AOT.add)
        nc.tensor.matmul(t_psr[:], t_wred[:], t_part[:])
        nc.scalar.copy(t_accf[:], t_psr[:])
        nc.gpsimd.dma_start(out=acc_out[:], in_=t_accf[:])

    _split_multi_waits(nc)
    return nc


def _host_consts(W1, b1, W2, b2, W3, b3):
    bd = np.float64
    beta32 = bd(np.float32(BETA))
    binv = 1.0 / beta32
    TH = 1.0

    # lhsT variant A (taus 0..6): spike rows were written with scale
    # TH*beta^-(tau+1), H-tilde wants beta^-tau * W * s -> W * beta / TH.
    # Variant B (tau 7): spike rows carry scale TH (written for the
    # phase-0 consumer), H-tilde wants beta^-7 * W * s.
    lhsT = np.zeros((128, 2 * 98), np.float32)
    for v, fac in ((0, beta32 / TH), (1, binv ** 7 / TH)):
        L = np.zeros((128, 98), np.float64)
        L[98:128, 0:64] = W1.T              # x rows are pre-scaled host-side
        L[0:64, 64:96] = fac * W2.T
        L[64:96, 96:98] = fac * W3.T
        lhsT[:, v * 98:(v + 1) * 98] = L.astype(np.float32)
    lhsT = lhsT.astype(np.float16)

    c1 = (bd(b1) / (1.0 - beta32)).astype(np.float32)
    c2 = (bd(b2) / (1.0 - beta32)).astype(np.float32)
    c3 = (bd(b3) / (1.0 - beta32)).astype(np.float32)
    th_main = np.concatenate([1.0 - c1, 1.0 - c2, 1.0 - c3]).astype(np.float32)

    # thr groups: chunk 0 (layers 2+3 frozen), chunk 1 (layer 3 frozen),
    # main; each group has the 8 inflated per-phase columns
    th = np.zeros((98, 3 * TC), np.float32)
    for g in range(3):
        base = th_main.copy()
        if g == 0:
            base[64:98] = BIG
        elif g == 1:
            base[96:98] = BIG
        for j in range(TC):
            col = (binv ** j) * bd(base)
            np.minimum(col, BIG, out=col)
            th[:, g * TC + j] = col.astype(np.float32)

    # Mt pre-init: chunk-0 tau-0 does u = beta*Mt + Ht, so Mt holds the
    # baseline's m0 (pre-decayed for the skewed layers 2/3)
    m0 = np.zeros(98, np.float64)
    m0[0:64] = -bd(c1)
    m0[64:96] = -bd(c2) * binv ** TC
    m0[96:98] = -bd(c3) * binv ** (2 * TC)
    m0b = np.ascontiguousarray(np.broadcast_to(
        m0.astype(np.float32)[:, None], (98, B))).astype(np.float16)

    # weighted reduce: stash partition p held phase tau = p % 8 spikes with
    # value TH*beta^-(tau+1) (tau<7) or TH (tau=7)
    wred = np.zeros((128, 1), np.float32)  # cast to f16 below
    for p in range(128):
        tau = p % TC
        scale = TH * binv ** (tau + 1) if tau < 7 else TH
        wred[p, 0] = np.float32(1.0 / scale)
    return lhsT, th, m0b, wred.astype(np.float16)


def kernel(x, W1, b1, W2, b2, W3, b3):
    from concourse.bass_utils import run_bass_kernel_spmd

    x = np.asarray(x, np.float32)
    W1 = np.asarray(W1, np.float32); b1 = np.asarray(b1, np.float32)
    W2 = np.asarray(W2, np.float32); b2 = np.asarray(b2, np.float32)
    W3 = np.asarray(W3, np.float32); b3 = np.asarray(b3, np.float32)

    if "nc" not in _cache:
        _cache["nc"] = _build_program()
    nc = _cache["nc"]

    lhsT, th, m0b, wred = _host_consts(W1, b1, W2, b2, W3, b3)
    binv32 = np.float32(1.0 / np.float64(np.float32(BETA)))
    presc = (binv32 ** np.arange(TC, dtype=np.float32))  # beta^-(tau%8)
    presc_t = np.tile(presc, T_FULL // TC)               # [1024]

    in_maps = []
    for core in range(N_CORES):
        xs = x[core * B:(core + 1) * B]                   # [256, 30, 1024]
        xt = np.transpose(xs, (1, 2, 0))                  # [30, 1024, 256]
        xt = (xt * presc_t[None, :, None]).astype(np.float16)
        in_maps.append({"xt": np.ascontiguousarray(xt), "lhsT": lhsT,
                        "th": th, "m0": m0b, "wred": wred})

    res = run_bass_kernel_spmd(nc, in_maps, list(range(N_CORES)))
    out = np.empty((B_FULL, 2), np.float32)
    for core in range(N_CORES):
        a = res.results[core]["acc"].reshape(2, B)        # [j, b]
        out[core * B:(core + 1) * B] = a.T
    # round: acc entries are sums of 1/scale-weighted fp16 spikes; the
    # true values are integers
    return np.rint(out).astype(np.float32)


# revision 3
# speedup vs baseline: 1.0772x; 1.0575x over previous
"""Trainium2 Bass kernel for nn_BinarySquareClassifier (3-layer LIF SNN).

Baseline skeleton (stacked 3-layer matmul, chunk-skew pipeline, serial
in-chunk scan) with a window-inflated fp16 scan:

- Within each 8-step chunk the membrane is kept inflated: Mt = beta^(-j)*M
  at in-chunk phase j. The decay then disappears from the recurrence:
      u(j)  = Mt(j-1) + Ht(j)          (tensor_tensor add, fp16, 2x)
      Mt(j) = u(j) - st(j-1)           (tensor_tensor sub, fp16, 2x)
      st(j) = (Mt(j) > thr_j) * imm_j  (tensor_scalar 2-op, fp16, 4x/Pool)
  where Ht = beta^(-j)*H comes out of the matmul directly (x staged with
  beta^(-tau) prescale; spike rows carry their own scale; lhsT absorbs
  the rest), thr_j = beta^(-j)*thr, and imm_j = beta^(-(j+1)) scales the
  written spike so the NEXT step's plain subtract applies the correctly
  inflated reset (j=7 writes scale 1.0 for the phase-0 consumer).
- At each chunk boundary the roll Mt *= beta^8 is folded into the tau=0
  update as a scalar_tensor_tensor (u = beta^8*Mt + Ht).
- The spike value also feeds the next chunk's matmul: W2/W3 lhsT blocks
  absorb the written scale (x beta/TH for tau<=6 consumers, x beta^-7/TH
  for the tau=7 column), so two lhsT variants cover a chunk.
- acc: layer-3 spike rows are stashed per chunk and reduced with a
  weighted-ones matmul that divides out the per-phase spike scales.
"""

import numpy as np
from contextlib import ExitStack

B_FULL, C_IN, T_FULL = 2048, 30, 1024
N_CORES = 8
B = B_FULL // N_CORES           # 256 batch per core
TC = 8                          # timesteps per chunk
WIN = 32                        # inflation window (4 chunks)
PAR = WIN // TC                 # chunk parities per window
N_CHUNKS = T_FULL // TC         # 128
BETA = 0.9
BIG = 3.0e4                     # fp16-safe "never fires" threshold

_cache = {}


def _split_multi_waits(nc):
    """This container's walrus accepts only ONE sync-wait per instruction;
    hoist extra waits onto same-engine NoOps inserted just before."""
    import concourse.mybir as mybir
    counter = 0
    for f in nc.m.functions:
        for blk in f.blocks:
            out = []
            changed = False
            for inst in blk.instructions:
                si = inst.sync_info
                if si is not None and si.on_wait is not None and len(si.on_wait) > 1:
                    waits = list(si.on_wait)
                    for w in waits[:-1]:
                        counter += 1
                        nop = mybir.InstNoOp(
                            name=f"waitsplit-{counter}", ins=[], outs=[])
                        nop.engine = inst.engine
                        nop.sync_info = mybir.SyncInfo(on_wait=[w], on_update=[])
                        out.append(nop)
                    inst.sync_info = mybir.SyncInfo(
                        on_wait=[waits[-1]], on_update=list(si.on_update or []))
                    changed = True
                out.append(inst)
            if changed:
                try:
                    blk.instructions[:] = out
                except TypeError:
                    blk.instructions = out


def _build_program():
    import concourse.bass as bass
    import concourse.mybir as mybir
    import concourse.tile as tile

    nc = bass.Bass("TRN2", target_bir_lowering=False, debug=False,
                   num_devices=N_CORES)
    f32 = mybir.dt.float32
    f16 = mybir.dt.float16
    AOT = mybir.AluOpType
    HA = B // 2

    binv = 1.0 / np.float64(np.float32(BETA))
    # spike write scale: written at window phase p, consumed (as reset) at
    # phase p+1, or phase 0 of the next window (p = WIN-1)
    imm = [float(np.float32(binv ** (p + 1))) if p < WIN - 1 else 1.0
           for p in range(WIN)]

    xt_in = nc.dram_tensor("xt", [C_IN, T_FULL, B], f16, kind="ExternalInput").ap()
    lhsT_in = nc.dram_tensor("lhsT", [128, 2 * 98], f16, kind="ExternalInput").ap()
    th_in = nc.dram_tensor("th", [98, (2 + PAR) * TC], f32, kind="ExternalInput").ap()
    m0_in = nc.dram_tensor("m0", [98, B], f16, kind="ExternalInput").ap()
    wred_in = nc.dram_tensor("wred", [128, 1], f16, kind="ExternalInput").ap()
    acc_out = nc.dram_tensor("acc", [1, 512], f32, kind="ExternalOutput").ap()

    with ExitStack() as ctx:
        tc = ctx.enter_context(tile.TileContext(nc))
        pool = ctx.enter_context(tc.tile_pool(name="sb", bufs=1))
        psum_pool = ctx.enter_context(tc.tile_pool(name="ps", bufs=1, space="PSUM"))

        t_lhsT = pool.tile([128, 2 * 98], f16, tag="lhsT", name="lhsT")
        t_th = pool.tile([98, (2 + PAR) * TC], f32, tag="th", name="th")
        t_M = pool.tile([98, B], f16, tag="M", name="M")
        t_u = pool.tile([98, B], f16, tag="u", name="u")
        t_rhs = [pool.tile([128, TC * B], f16, tag=f"rhs{i}", name=f"rhs{i}")
                 for i in range(2)]
        t_stash = pool.tile([128, 4096], f16, tag="stash", name="stash")
        t_wred = pool.tile([128, 1], f16, tag="wred", name="wred")
        t_part = pool.tile([128, 512], f16, tag="part", name="part")
        t_accf = pool.tile([1, 512], f32, tag="accf", name="accf")
        t_ps = [psum_pool.tile([98, TC * B], f32, tag=f"H{i}", name=f"H{i}")
                for i in range(2)]
        t_hs = [pool.tile([98, TC * B], f16, tag=f"Hs{i}", name=f"Hs{i}")
                for i in range(2)]
        t_psr = t_ps[1][0:1, 0:512]  # reuse a PSUM bank for the final reduce

        nc.sync.dma_start(out=t_lhsT[:], in_=lhsT_in[:])
        nc.sync.dma_start(out=t_th[:], in_=th_in[:])
        nc.sync.dma_start(out=t_M[:], in_=m0_in[:])
        nc.sync.dma_start(out=t_wred[:], in_=wred_in[:])
        # Only rhs0's spike rows need zeroing (rhs1's are fully written by
        # chunk 0's scan before anything reads them; rows 0:98 also cover
        # the tau=0 reset read of the last column). Split per matmul block
        # so the first sub-matmul isn't gated on the whole memset.
        for blk in range(4):
            nc.gpsimd.memset(t_rhs[0][0:98, blk * 512:(blk + 1) * 512], 0.0)

        # x prefetch for chunk 0, split per matmul block
        for blk in range(4):
            nc.sync.dma_start(
                out=t_rhs[0][98:128, blk * 512:(blk + 1) * 512],
                in_=xt_in[:, blk * 2:(blk + 1) * 2, :].rearrange(
                    "c t b -> c (t b)"),
            )

        def th_group(c):
            # 0/1: warmup guards (chunks 0,1); then per-parity main groups
            return c if c < 2 else 2 + (c % PAR)

        for c in range(N_CHUNKS + 2):
            cur = t_rhs[c % 2]
            nxt = t_rhs[(c + 1) % 2]
            ps = t_ps[c % 2]
            hs = t_hs[c % 2]
            tg = th_group(c)

            # prefetch x for chunk c+1 (overlaps this chunk's scan)
            if c + 1 < N_CHUNKS:
                nc.sync.dma_start(
                    out=nxt[98:128, :],
                    in_=xt_in[:, (c + 1) * TC:(c + 2) * TC, :].rearrange(
                        "c t b -> c (t b)"),
                )

            # stacked matmul. Spike rows written at window phase p carry
            # scale beta^-(p+1) (or 1.0 at p=WIN-1). For chunks at parity
            # q>0 every column needs lhsT fac beta^-7; at parity 0, cols
            # 0..6 (consumers of late-prev-window spikes) need
            # beta^(WIN-7) and col 7 needs beta^-7.
            if c % PAR == 0:
                mm_blocks = [(0, 512, 0), (512, 1024, 0), (1024, 1536, 0),
                             (1536, 1792, 0), (1792, 2048, 98)]
            else:
                mm_blocks = [(0, 512, 98), (512, 1024, 98),
                             (1024, 1536, 98), (1536, 2048, 98)]
            for lo, hi, lv in mm_blocks:
                nc.tensor.matmul(ps[:, lo:hi], t_lhsT[:, lv:lv + 98],
                                 cur[0:128, lo:hi])
                nc.scalar.copy(hs[:, lo:hi], ps[:, lo:hi])

            # serial inflated scan; the window roll only happens at
            # parity-0 chunk boundaries
            for tau in range(TC):
                if tau == 0:
                    r_ap = cur[0:98, (TC - 1) * B:TC * B]
                else:
                    r_ap = nxt[0:98, (tau - 1) * B:tau * B]
                h0 = tau * B
                phase = (c % PAR) * TC + tau
                th = t_th[:, tg * TC + tau:tg * TC + tau + 1]

                if tau == 0 and c % PAR == 0:
                    # window roll folded in: u = roll*Mt + Ht
                    roll = float(np.float32(BETA)) if c == 0 else \
                        float(np.float32(np.float64(np.float32(BETA)) ** WIN))
                    def upd(lo, hi):
                        nc.vector.scalar_tensor_tensor(
                            t_u[:, lo:hi], t_M[:, lo:hi], roll,
                            hs[:, h0 + lo:h0 + hi], AOT.mult, AOT.add)
                else:
                    def upd(lo, hi):
                        nc.vector.tensor_tensor(
                            t_u[:, lo:hi], t_M[:, lo:hi],
                            hs[:, h0 + lo:h0 + hi], AOT.add)

                def tt(lo, hi):
                    nc.vector.tensor_tensor(
                        t_M[:, lo:hi], t_u[:, lo:hi], r_ap[:, lo:hi],
                        AOT.subtract)

                def spike(eng, lo, hi):
                    eng.tensor_scalar(
                        nxt[0:98, tau * B + lo:tau * B + hi],
                        t_M[:, lo:hi], th, imm[phase], AOT.is_gt, AOT.mult)

                upd(0, HA)
                upd(HA, B)
                tt(0, HA)
                tt(HA, B)
                spike(nc.vector, 0, B)

            # stash layer-3 spike rows for frame f = c-2
            f = c - 2
            if 0 <= f < N_CHUNKS:
                p0 = (f % 16) * TC
                cb = f // 16
                for jr in range(2):
                    nc.sync.dma_start(
                        out=t_stash[p0:p0 + TC,
                                    cb * 512 + jr * B:cb * 512 + (jr + 1) * B],
                        in_=nxt[96 + jr:97 + jr, :],
                    )

        # reduce stash: sum the 8 column blocks (fp32 accumulate), then a
        # weighted-ones matmul divides out the per-phase spike scales
        nc.vector.tensor_tensor(
            t_part[:], t_stash[:, 0:512], t_stash[:, 512:1024], AOT.add)
        for cb in range(2, 8):
            nc.vector.tensor_tensor(
                t_part[:], t_part[:], t_stash[:, cb * 512:(cb + 1) * 512],
                AOT.add)
        nc.tensor.matmul(t_psr[:], t_wred[:], t_part[:])
        nc.scalar.copy(t_accf[:], t_psr[:])
        nc.gpsimd.dma_start(out=acc_out[:], in_=t_accf[:])

    _split_multi_waits(nc)
    return nc


def _host_consts(W1, b1, W2, b2, W3, b3):
    bd = np.float64
    beta32 = bd(np.float32(BETA))
    binv = 1.0 / beta32
    TH = 1.0

    # lhsT variant A (parity-0 chunks, cols 0..6): spikes written late in
    # the previous window with scale beta^-(wp+1), wp = WIN-8+tau; wanted
    # beta^-tau -> fac beta^(WIN-7). Variant B (everything else): fac
    # beta^-7 (uniform; includes the window-boundary col via imm=1.0).
    lhsT = np.zeros((128, 2 * 98), np.float32)
    for v, fac in ((0, np.float64(np.float32(BETA)) ** (WIN - 7) / TH),
                   (1, binv ** 7 / TH)):
        L = np.zeros((128, 98), np.float64)
        L[98:128, 0:64] = W1.T              # x rows are pre-scaled host-side
        L[0:64, 64:96] = fac * W2.T
        L[64:96, 96:98] = fac * W3.T
        lhsT[:, v * 98:(v + 1) * 98] = L.astype(np.float32)
    lhsT = lhsT.astype(np.float16)

    c1 = (bd(b1) / (1.0 - beta32)).astype(np.float32)
    c2 = (bd(b2) / (1.0 - beta32)).astype(np.float32)
    c3 = (bd(b3) / (1.0 - beta32)).astype(np.float32)
    th_main = np.concatenate([1.0 - c1, 1.0 - c2, 1.0 - c3]).astype(np.float32)

    # thr groups: chunk 0 (layers 2+3 frozen, phases 0..7), chunk 1
    # (layer 3 frozen, phases 8..15), then one group per chunk parity
    th = np.zeros((98, (2 + PAR) * TC), np.float32)
    for g in range(2 + PAR):
        base = th_main.copy()
        if g == 0:
            base[64:98] = BIG
        elif g == 1:
            base[96:98] = BIG
        p0 = g * TC if g < 2 else (g - 2) * TC
        for j in range(TC):
            col = (binv ** (p0 + j)) * bd(base)
            np.minimum(col, BIG, out=col)
            th[:, g * TC + j] = col.astype(np.float32)

    # Mt pre-init: chunk-0 tau-0 does u = beta*Mt + Ht, so Mt holds the
    # baseline's m0 (pre-decayed for the skewed layers 2/3)
    # layer l joins at chunk l-1 (window phase 8*(l-1)); no rolls happen
    # before then inside window 0, so pre-divide by beta^(phase) (plus one
    # beta for the chunk-0 fold's roll=beta)
    m0 = np.zeros(98, np.float64)
    m0[0:64] = -bd(c1)
    m0[64:96] = -bd(c2) * binv ** TC
    m0[96:98] = -bd(c3) * binv ** (2 * TC)
    m0b = np.ascontiguousarray(np.broadcast_to(
        m0.astype(np.float32)[:, None], (98, B))).astype(np.float16)

    # weighted reduce: stash partition p = (f%16)*8 + tau holds frame-f
    # spikes, which chunk c = f+2's scan wrote at window phase
    # ((f+2)%PAR)*8 + tau with value TH*beta^-(ph+1) (TH at the boundary)
    wred = np.zeros((128, 1), np.float32)  # cast to f16 below
    for p in range(128):
        tau = p % TC
        fmod = (p // TC + 2) % PAR
        ph = fmod * TC + tau
        scale = TH * binv ** (ph + 1) if ph < WIN - 1 else TH
        wred[p, 0] = np.float32(1.0 / scale)
    return lhsT, th, m0b, wred.astype(np.float16)


def kernel(x, W1, b1, W2, b2, W3, b3):
    from concourse.bass_utils import run_bass_kernel_spmd

    x = np.asarray(x, np.float32)
    W1 = np.asarray(W1, np.float32); b1 = np.asarray(b1, np.float32)
    W2 = np.asarray(W2, np.float32); b2 = np.asarray(b2, np.float32)
    W3 = np.asarray(W3, np.float32); b3 = np.asarray(b3, np.float32)

    if "nc" not in _cache:
        _cache["nc"] = _build_program()
    nc = _cache["nc"]

    lhsT, th, m0b, wred = _host_consts(W1, b1, W2, b2, W3, b3)
    binv32 = np.float32(1.0 / np.float64(np.float32(BETA)))
    presc = (binv32 ** np.arange(WIN, dtype=np.float32))  # beta^-(t%WIN)
    presc_t = np.tile(presc, T_FULL // WIN)               # [1024]

    in_maps = []
    for core in range(N_CORES):
        xs = x[core * B:(core + 1) * B]                   # [256, 30, 1024]
        xt = np.transpose(xs, (1, 2, 0))                  # [30, 1024, 256]
        xt = (xt * presc_t[None, :, None]).astype(np.float16)
        in_maps.append({"xt": np.ascontiguousarray(xt), "lhsT": lhsT,
                        "th": th, "m0": m0b, "wred": wred})

    res = run_bass_kernel_spmd(nc, in_maps, list(range(N_CORES)))
    out = np.empty((B_FULL, 2), np.float32)
    for core in range(N_CORES):
        a = res.results[core]["acc"].reshape(2, B)        # [j, b]
        out[core * B:(core + 1) * B] = a.T
    # round: acc entries are sums of 1/scale-weighted fp16 spikes; the
    # true values are integers
    return np.rint(out).astype(np.float32)


# revision 4
# speedup vs baseline: 1.0824x; 1.0049x over previous
"""Trainium2 Bass kernel for nn_BinarySquareClassifier (3-layer LIF SNN).

Baseline skeleton (stacked 3-layer matmul, chunk-skew pipeline, serial
in-chunk scan) with a window-inflated fp16 scan:

- Within each 8-step chunk the membrane is kept inflated: Mt = beta^(-j)*M
  at in-chunk phase j. The decay then disappears from the recurrence:
      u(j)  = Mt(j-1) + Ht(j)          (tensor_tensor add, fp16, 2x)
      Mt(j) = u(j) - st(j-1)           (tensor_tensor sub, fp16, 2x)
      st(j) = (Mt(j) > thr_j) * imm_j  (tensor_scalar 2-op, fp16, 4x/Pool)
  where Ht = beta^(-j)*H comes out of the matmul directly (x staged with
  beta^(-tau) prescale; spike rows carry their own scale; lhsT absorbs
  the rest), thr_j = beta^(-j)*thr, and imm_j = beta^(-(j+1)) scales the
  written spike so the NEXT step's plain subtract applies the correctly
  inflated reset (j=7 writes scale 1.0 for the phase-0 consumer).
- At each chunk boundary the roll Mt *= beta^8 is folded into the tau=0
  update as a scalar_tensor_tensor (u = beta^8*Mt + Ht).
- The spike value also feeds the next chunk's matmul: W2/W3 lhsT blocks
  absorb the written scale (x beta/TH for tau<=6 consumers, x beta^-7/TH
  for the tau=7 column), so two lhsT variants cover a chunk.
- acc: layer-3 spike rows are stashed per chunk and reduced with a
  weighted-ones matmul that divides out the per-phase spike scales.
"""

import numpy as np
from contextlib import ExitStack

B_FULL, C_IN, T_FULL = 2048, 30, 1024
N_CORES = 8
B = B_FULL // N_CORES           # 256 batch per core
TC = 8                          # timesteps per chunk
WIN = 32                        # inflation window (4 chunks)
PAR = WIN // TC                 # chunk parities per window
N_CHUNKS = T_FULL // TC         # 128
BETA = 0.9
BIG = 3.0e4                     # fp16-safe "never fires" threshold

_cache = {}


def _split_multi_waits(nc):
    """This container's walrus accepts only ONE sync-wait per instruction;
    hoist extra waits onto same-engine NoOps inserted just before."""
    import concourse.mybir as mybir
    counter = 0
    for f in nc.m.functions:
        for blk in f.blocks:
            out = []
            changed = False
            for inst in blk.instructions:
                si = inst.sync_info
                if si is not None and si.on_wait is not None and len(si.on_wait) > 1:
                    waits = list(si.on_wait)
                    for w in waits[:-1]:
                        counter += 1
                        nop = mybir.InstNoOp(
                            name=f"waitsplit-{counter}", ins=[], outs=[])
                        nop.engine = inst.engine
                        nop.sync_info = mybir.SyncInfo(on_wait=[w], on_update=[])
                        out.append(nop)
                    inst.sync_info = mybir.SyncInfo(
                        on_wait=[waits[-1]], on_update=list(si.on_update or []))
                    changed = True
                out.append(inst)
            if changed:
                try:
                    blk.instructions[:] = out
                except TypeError:
                    blk.instructions = out


def _build_program():
    import concourse.bass as bass
    import concourse.mybir as mybir
    import concourse.tile as tile

    nc = bass.Bass("TRN2", target_bir_lowering=False, debug=False,
                   num_devices=N_CORES)
    f32 = mybir.dt.float32
    f16 = mybir.dt.float16
    AOT = mybir.AluOpType
    PS = 32                       # pool-owned batch slice
    DW = B - PS                   # DVE-owned width
    HA = DW // 2

    binv = 1.0 / np.float64(np.float32(BETA))
    # spike write scale: written at window phase p, consumed (as reset) at
    # phase p+1, or phase 0 of the next window (p = WIN-1)
    imm = [float(np.float32(binv ** (p + 1))) if p < WIN - 1 else 1.0
           for p in range(WIN)]

    xt_in = nc.dram_tensor("xt", [C_IN, T_FULL, B], f16, kind="ExternalInput").ap()
    lhsT_in = nc.dram_tensor("lhsT", [128, 2 * 98], f16, kind="ExternalInput").ap()
    th_in = nc.dram_tensor("th", [98, (2 + PAR) * TC], f32, kind="ExternalInput").ap()
    m0_in = nc.dram_tensor("m0", [98, B], f16, kind="ExternalInput").ap()
    wred_in = nc.dram_tensor("wred", [128, 1], f16, kind="ExternalInput").ap()
    acc_out = nc.dram_tensor("acc", [1, 512], f32, kind="ExternalOutput").ap()

    with ExitStack() as ctx:
        tc = ctx.enter_context(tile.TileContext(nc))
        pool = ctx.enter_context(tc.tile_pool(name="sb", bufs=1))
        psum_pool = ctx.enter_context(tc.tile_pool(name="ps", bufs=1, space="PSUM"))

        t_lhsT = pool.tile([128, 2 * 98], f16, tag="lhsT", name="lhsT")
        t_th = pool.tile([98, (2 + PAR) * TC], f32, tag="th", name="th")
        t_M = pool.tile([98, B], f16, tag="M", name="M")
        t_u = pool.tile([98, B], f16, tag="u", name="u")
        t_rhs = [pool.tile([128, TC * B], f16, tag=f"rhs{i}", name=f"rhs{i}")
                 for i in range(2)]
        t_stash = pool.tile([128, 4096], f16, tag="stash", name="stash")
        t_wred = pool.tile([128, 1], f16, tag="wred", name="wred")
        t_part = pool.tile([128, 512], f16, tag="part", name="part")
        t_accf = pool.tile([1, 512], f32, tag="accf", name="accf")
        t_ps = [psum_pool.tile([98, TC * B], f32, tag=f"H{i}", name=f"H{i}")
                for i in range(2)]
        t_hs = [pool.tile([98, TC * B], f16, tag=f"Hs{i}", name=f"Hs{i}")
                for i in range(2)]
        t_psr = t_ps[1][0:1, 0:512]  # reuse a PSUM bank for the final reduce

        nc.sync.dma_start(out=t_lhsT[:], in_=lhsT_in[:])
        nc.sync.dma_start(out=t_th[:], in_=th_in[:])
        nc.sync.dma_start(out=t_M[:], in_=m0_in[:])
        nc.sync.dma_start(out=t_wred[:], in_=wred_in[:])
        # Only rhs0's spike rows need zeroing (rhs1's are fully written by
        # chunk 0's scan before anything reads them; rows 0:98 also cover
        # the tau=0 reset read of the last column). Split per matmul block
        # so the first sub-matmul isn't gated on the whole memset.
        for blk in range(4):
            nc.gpsimd.memset(t_rhs[0][0:98, blk * 512:(blk + 1) * 512], 0.0)

        # x prefetch for chunk 0, split per matmul block
        for blk in range(4):
            nc.sync.dma_start(
                out=t_rhs[0][98:128, blk * 512:(blk + 1) * 512],
                in_=xt_in[:, blk * 2:(blk + 1) * 2, :].rearrange(
                    "c t b -> c (t b)"),
            )

        def th_group(c):
            # 0/1: warmup guards (chunks 0,1); then per-parity main groups
            return c if c < 2 else 2 + (c % PAR)

        for c in range(N_CHUNKS + 2):
            cur = t_rhs[c % 2]
            nxt = t_rhs[(c + 1) % 2]
            ps = t_ps[c % 2]
            hs = t_hs[c % 2]
            tg = th_group(c)

            # prefetch x for chunk c+1 (overlaps this chunk's scan)
            if c + 1 < N_CHUNKS:
                nc.sync.dma_start(
                    out=nxt[98:128, :],
                    in_=xt_in[:, (c + 1) * TC:(c + 2) * TC, :].rearrange(
                        "c t b -> c (t b)"),
                )

            # stacked matmul. Spike rows written at window phase p carry
            # scale beta^-(p+1) (or 1.0 at p=WIN-1). For chunks at parity
            # q>0 every column needs lhsT fac beta^-7; at parity 0, cols
            # 0..6 (consumers of late-prev-window spikes) need
            # beta^(WIN-7) and col 7 needs beta^-7.
            if c % PAR == 0:
                mm_blocks = [(0, 512, 0), (512, 1024, 0), (1024, 1536, 0),
                             (1536, 1792, 0), (1792, 2048, 98)]
            else:
                mm_blocks = [(0, 512, 98), (512, 1024, 98),
                             (1024, 1536, 98), (1536, 2048, 98)]
            for lo, hi, lv in mm_blocks:
                nc.tensor.matmul(ps[:, lo:hi], t_lhsT[:, lv:lv + 98],
                                 cur[0:128, lo:hi])
                nc.scalar.copy(hs[:, lo:hi], ps[:, lo:hi])

            # serial inflated scan; the window roll only happens at
            # parity-0 chunk boundaries
            for tau in range(TC):
                if tau == 0:
                    r_ap = cur[0:98, (TC - 1) * B:TC * B]
                else:
                    r_ap = nxt[0:98, (tau - 1) * B:tau * B]
                h0 = tau * B
                phase = (c % PAR) * TC + tau
                th = t_th[:, tg * TC + tau:tg * TC + tau + 1]

                if tau == 0 and c % PAR == 0:
                    # window roll folded in: u = roll*Mt + Ht
                    roll = float(np.float32(BETA)) if c == 0 else \
                        float(np.float32(np.float64(np.float32(BETA)) ** WIN))
                    def upd(lo, hi, eng=None):
                        (eng or nc.vector).scalar_tensor_tensor(
                            t_u[:, lo:hi], t_M[:, lo:hi], roll,
                            hs[:, h0 + lo:h0 + hi], AOT.mult, AOT.add)
                else:
                    def upd(lo, hi, eng=None):
                        (eng or nc.vector).tensor_tensor(
                            t_u[:, lo:hi], t_M[:, lo:hi],
                            hs[:, h0 + lo:h0 + hi], AOT.add)

                def tt(lo, hi, eng=None):
                    (eng or nc.vector).tensor_tensor(
                        t_M[:, lo:hi], t_u[:, lo:hi], r_ap[:, lo:hi],
                        AOT.subtract)

                def spike(eng, lo, hi):
                    eng.tensor_scalar(
                        nxt[0:98, tau * B + lo:tau * B + hi],
                        t_M[:, lo:hi], th, imm[phase], AOT.is_gt, AOT.mult)

                upd(0, HA)
                upd(HA, DW)
                if tau == 0 and c % PAR == 0:
                    upd(DW, B)          # Pool cannot run stt; roll on DVE
                else:
                    upd(DW, B, nc.gpsimd)
                tt(0, HA)
                tt(HA, DW)
                tt(DW, B, nc.gpsimd)
                spike(nc.vector, 0, DW)
                spike(nc.gpsimd, DW, B)

            # stash layer-3 spike rows for frame f = c-2
            f = c - 2
            if 0 <= f < N_CHUNKS:
                p0 = (f % 16) * TC
                cb = f // 16
                for jr in range(2):
                    nc.sync.dma_start(
                        out=t_stash[p0:p0 + TC,
                                    cb * 512 + jr * B:cb * 512 + (jr + 1) * B],
                        in_=nxt[96 + jr:97 + jr, :],
                    )

        # reduce stash: sum the 8 column blocks (fp32 accumulate), then a
        # weighted-ones matmul divides out the per-phase spike scales
        nc.vector.tensor_tensor(
            t_part[:], t_stash[:, 0:512], t_stash[:, 512:1024], AOT.add)
        for cb in range(2, 8):
            nc.vector.tensor_tensor(
                t_part[:], t_part[:], t_stash[:, cb * 512:(cb + 1) * 512],
                AOT.add)
        nc.tensor.matmul(t_psr[:], t_wred[:], t_part[:])
        nc.scalar.copy(t_accf[:], t_psr[:])
        nc.gpsimd.dma_start(out=acc_out[:], in_=t_accf[:])

    _split_multi_waits(nc)
    return nc


def _host_consts(W1, b1, W2, b2, W3, b3):
    bd = np.float64
    beta32 = bd(np.float32(BETA))
    binv = 1.0 / beta32
    TH = 1.0

    # lhsT variant A (parity-0 chunks, cols 0..6): spikes written late in
    # the previous window with scale beta^-(wp+1), wp = WIN-8+tau; wanted
    # beta^-tau -> fac beta^(WIN-7). Variant B (everything else): fac
    # beta^-7 (uniform; includes the window-boundary col via imm=1.0).
    lhsT = np.zeros((128, 2 * 98), np.float32)
    for v, fac in ((0, np.float64(np.float32(BETA)) ** (WIN - 7) / TH),
                   (1, binv ** 7 / TH)):
        L = np.zeros((128, 98), np.float64)
        L[98:128, 0:64] = W1.T              # x rows are pre-scaled host-side
        L[0:64, 64:96] = fac * W2.T
        L[64:96, 96:98] = fac * W3.T
        lhsT[:, v * 98:(v + 1) * 98] = L.astype(np.float32)
    lhsT = lhsT.astype(np.float16)

    c1 = (bd(b1) / (1.0 - beta32)).astype(np.float32)
    c2 = (bd(b2) / (1.0 - beta32)).astype(np.float32)
    c3 = (bd(b3) / (1.0 - beta32)).astype(np.float32)
    th_main = np.concatenate([1.0 - c1, 1.0 - c2, 1.0 - c3]).astype(np.float32)

    # thr groups: chunk 0 (layers 2+3 frozen, phases 0..7), chunk 1
    # (layer 3 frozen, phases 8..15), then one group per chunk parity
    th = np.zeros((98, (2 + PAR) * TC), np.float32)
    for g in range(2 + PAR):
        base = th_main.copy()
        if g == 0:
            base[64:98] = BIG
        elif g == 1:
            base[96:98] = BIG
        p0 = g * TC if g < 2 else (g - 2) * TC
        for j in range(TC):
            col = (binv ** (p0 + j)) * bd(base)
            np.minimum(col, BIG, out=col)
            th[:, g * TC + j] = col.astype(np.float32)

    # Mt pre-init: chunk-0 tau-0 does u = beta*Mt + Ht, so Mt holds the
    # baseline's m0 (pre-decayed for the skewed layers 2/3)
    # layer l joins at chunk l-1 (window phase 8*(l-1)); no rolls happen
    # before then inside window 0, so pre-divide by beta^(phase) (plus one
    # beta for the chunk-0 fold's roll=beta)
    m0 = np.zeros(98, np.float64)
    m0[0:64] = -bd(c1)
    m0[64:96] = -bd(c2) * binv ** TC
    m0[96:98] = -bd(c3) * binv ** (2 * TC)
    m0b = np.ascontiguousarray(np.broadcast_to(
        m0.astype(np.float32)[:, None], (98, B))).astype(np.float16)

    # weighted reduce: stash partition p = (f%16)*8 + tau holds frame-f
    # spikes, which chunk c = f+2's scan wrote at window phase
    # ((f+2)%PAR)*8 + tau with value TH*beta^-(ph+1) (TH at the boundary)
    wred = np.zeros((128, 1), np.float32)  # cast to f16 below
    for p in range(128):
        tau = p % TC
        fmod = (p // TC + 2) % PAR
        ph = fmod * TC + tau
        scale = TH * binv ** (ph + 1) if ph < WIN - 1 else TH
        wred[p, 0] = np.float32(1.0 / scale)
    return lhsT, th, m0b, wred.astype(np.float16)


def kernel(x, W1, b1, W2, b2, W3, b3):
    from concourse.bass_utils import run_bass_kernel_spmd

    x = np.asarray(x, np.float32)
    W1 = np.asarray(W1, np.float32); b1 = np.asarray(b1, np.float32)
    W2 = np.asarray(W2, np.float32); b2 = np.asarray(b2, np.float32)
    W3 = np.asarray(W3, np.float32); b3 = np.asarray(b3, np.float32)

    if "nc" not in _cache:
        _cache["nc"] = _build_program()
    nc = _cache["nc"]

    lhsT, th, m0b, wred = _host_consts(W1, b1, W2, b2, W3, b3)
    binv32 = np.float32(1.0 / np.float64(np.float32(BETA)))
    presc = (binv32 ** np.arange(WIN, dtype=np.float32))  # beta^-(t%WIN)
    presc_t = np.tile(presc, T_FULL // WIN)               # [1024]

    in_maps = []
    for core in range(N_CORES):
        xs = x[core * B:(core + 1) * B]                   # [256, 30, 1024]
        xt = np.transpose(xs, (1, 2, 0))                  # [30, 1024, 256]
        xt = (xt * presc_t[None, :, None]).astype(np.float16)
        in_maps.append({"xt": np.ascontiguousarray(xt), "lhsT": lhsT,
                        "th": th, "m0": m0b, "wred": wred})

    res = run_bass_kernel_spmd(nc, in_maps, list(range(N_CORES)))
    out = np.empty((B_FULL, 2), np.float32)
    for core in range(N_CORES):
        a = res.results[core]["acc"].reshape(2, B)        # [j, b]
        out[core * B:(core + 1) * B] = a.T
    # round: acc entries are sums of 1/scale-weighted fp16 spikes; the
    # true values are integers
    return np.rint(out).astype(np.float32)
